# revision 23
# baseline (speedup 1.0000x reference)
"""Trainium2 Bass kernel for per-sample conv self-attention.

Reference computation (per batch sample b, N = H*W = 4096, C = 64, C8 = 8):
    q = x @ wq + bq            [N, 8]
    k = x @ wk + bk            [N, 8]
    v = x @ wv + bv            [N, 64]
    attn = softmax(q @ k^T)    [N, N]   (softmax over keys, no scaling)
    out  = attn @ v * gamma + x

Sharding: data-parallel over batch - 8 samples onto 8 NeuronCores, one
sample per core.  Inside a core the attention matrix is processed
flash-style (never materialized in HBM).

The kernel is ACT(exp)-bound: 16.8M exps on ScalarE at 1 elem/cycle/
partition (measured in-stream cost (N+171)/1.2 ns per tile) gives a
~122us floor for the exp stream, so the schedule keeps ScalarE
saturated from ~10us onward:

  * x arrives via split DMAs on two HWDGE queues (sync + scalar) so the
    first transpose chunks land early.
  * The first window interleaves n-blocks 0 and 1 (22 group slots) so
    only quarter 0 of q^T is needed up front and k^T chunk deadlines are
    halved; preamble quarters 1-3 (PE transposes of x^T, q/k/v
    projections) are emitted in small phases between those slots,
    ping-ponging between the spare "aux" PSUM bank and the (not yet
    used) oacc bank.
  * PV (out^T accumulation) is lagged behind the exp stream via a
    budgeted work queue: zero PV during the preamble window, then 4
    chunks/slot until it catches up (by ~nb4).  This frees the PE for
    preamble work while ScalarE streams exp.
  * S^T blocks are computed with K=8 matmuls packed 3-up into the PE
    array via row groups (partition offsets 0/32/64) into a 6-bank PSUM
    ring; exp() runs PSUM->SBUF (bf16) with free-dim 1536.  No row-max
    subtraction: |S| <= ~30 stays in range.
  * out^T [65, n] accumulates in PSUM as v'.T @ E^T (ones column of v'
    gives the softmax denominator in row 64).  Finale per 128-query
    chunk: PE-transpose (via aux bank) then VectorE computes
    y = out * (gamma/denom) + x and DMAs out.  ScalarE does only exp.
"""

from contextlib import ExitStack

import numpy as np

import concourse.bass as bass
import concourse.mybir as mybir
import concourse.tile as tile
from concourse import bacc
from concourse.bass_utils import run_bass_kernel_spmd
from concourse.masks import make_identity

F32 = mybir.dt.float32
BF16 = mybir.dt.bfloat16
AF = mybir.ActivationFunctionType
ALU = mybir.AluOpType

B, H, W, C = 8, 64, 64, 64
N = H * W          # 4096 pixels (queries == keys)
C8 = C // 8        # 8  qk head dim
NB = 8             # n blocks
NBLK = N // NB     # 512 queries per block
MCH = N // 128     # 32 m-chunks of 128 keys
# m-chunks per group: 3 fit the 3 usable PE row-group replicas and give
# exp() free-dim 1536 (3 PSUM banks), double buffered: 6 banks + 1 out + 1 aux
GROUP_SIZES = [3] * 10 + [2]   # sums to 32


def _body(nc, tc, io):
    x_d, wq_d, bq_d, wk_d, bk_d, wv_d, bv_d, gamma_d, y_d = io

    # ---------------- persistent SBUF tensors ----------------
    ctx = ExitStack()
    singles = ctx.enter_context(tc.tile_pool(name="singles", bufs=1))
    ident = singles.tile([128, 128], BF16)       # identity for PE transposes
    x_sb = singles.tile([128, MCH * C], F32)     # resident x, chunk j at cols 64j
    xb = singles.tile([128, MCH * C], BF16)      # bf16 copy of x
    xT = singles.tile([C + 1, N], BF16)          # x^T with ones row 64
    qT_rep = singles.tile([128, N], BF16)        # q^T replicated at parts 0/32/64
    kT_rep = singles.tile([128, N], BF16)        # k^T replicated at parts 0/32/64
    v_all = singles.tile([128, MCH * (C + 1)], BF16)  # v'_j at cols 65j, ones col 64
    gamma_sb = singles.tile([128, 1], F32)
    wq_st = singles.tile([C + 1, C8], F32)
    wk_st = singles.tile([C + 1, C8], F32)
    wv_st = singles.tile([C + 1, C], F32)
    wqp = singles.tile([C + 1, 128], BF16)       # wq' replicated into cols 0/32/64
    wkp = singles.tile([C + 1, 128], BF16)
    wvp = singles.tile([C + 1, C], BF16)

    make_identity(nc, ident)

    # -------- input DMAs: x first (critical path), split across queues -----
    # x quarter 0 in 4 pieces alternating between the two HWDGE queues so
    # chunks 0-1 land as early as possible; quarters 1-3 split per engine.
    for h4 in range(4):
        eng = nc.sync if h4 % 2 == 0 else nc.scalar
        eng.dma_start(
            out=x_sb[:, 128 * h4 : 128 * (h4 + 1)].rearrange(
                "p (c f) -> p c f", f=C
            ),
            in_=x_d[256 * h4 : 256 * (h4 + 1), :].rearrange(
                "(c p) f -> p c f", p=128
            ),
        )
    # quarters 1-3 in halves, one half per queue, so both descriptor
    # pipelines stream x concurrently
    for r in range(1, 4):
        for h2 in range(2):
            eng = nc.sync if h2 == 0 else nc.scalar
            c0 = 512 * r + 256 * h2
            eng.dma_start(
                out=x_sb[:, c0 : c0 + 256].rearrange("p (c f) -> p c f", f=C),
                in_=x_d[2 * c0 : 2 * c0 + 512, :].rearrange(
                    "(c p) f -> p c f", p=128
                ),
            )
    # fast contiguous memsets first (a strided ones-memset measured 3.5us on
    # GpSimd and stalled VectorE via the shared SBUF port).  v_all is set to
    # all-ones; the v projections later overwrite cols 0-63 of each chunk,
    # leaving the ones column 64 (the softmax-denominator trick).
    nc.gpsimd.memset(wqp[:], 0.0)
    nc.gpsimd.memset(wkp[:], 0.0)
    nc.gpsimd.memset(v_all[:], 1.0)
    nc.gpsimd.memset(xT[C : C + 1, :], 1.0)
    # weights via SWDGE (gpsimd)
    nc.gpsimd.dma_start(out=wq_st[0:C, :], in_=wq_d)
    nc.gpsimd.dma_start(out=wk_st[0:C, :], in_=wk_d)
    nc.gpsimd.dma_start(out=wv_st[0:C, :], in_=wv_d)
    nc.gpsimd.dma_start(out=wq_st[C : C + 1, :], in_=bq_d)
    nc.gpsimd.dma_start(out=wk_st[C : C + 1, :], in_=bk_d)
    nc.gpsimd.dma_start(out=wv_st[C : C + 1, :], in_=bv_d)
    nc.gpsimd.dma_start(out=gamma_sb[:], in_=gamma_d.to_broadcast((128, 1)))

    # ---------------- PSUM pools (persistent; 6 + 1 + 1 banks) -------------
    st_pool = ctx.enter_context(tc.tile_pool(name="st", bufs=2, space="PSUM"))
    out_pool = ctx.enter_context(tc.tile_pool(name="oacc", bufs=1, space="PSUM"))
    aux_pool = ctx.enter_context(tc.tile_pool(name="aux", bufs=1, space="PSUM"))
    aux = aux_pool.tile([128, 512], F32)  # one spare bank

    et_pool = ctx.enter_context(tc.tile_pool(name="et", bufs=15))
    ob_pool = ctx.enter_context(tc.tile_pool(name="ob", bufs=2))
    yt_pool = ctx.enter_context(tc.tile_pool(name="yt", bufs=2))
    fin_pool = ctx.enter_context(tc.tile_pool(name="fin", bufs=2))

    # second preamble scratch bank: borrow the (yet unused) oacc bank
    aux2 = out_pool.tile([128, 512], F32, tag="oacc", name="aux2")

    def xb_cast(c0, w):
        nc.vector.tensor_copy(out=xb[:, c0 : c0 + w], in_=x_sb[:, c0 : c0 + w])

    def stage_weights():
        for i in range(3):
            nc.vector.tensor_copy(out=wqp[:, 32 * i : 32 * i + C8], in_=wq_st[:])
            nc.vector.tensor_copy(out=wkp[:, 32 * i : 32 * i + C8], in_=wk_st[:])
        nc.vector.tensor_copy(out=wvp[:], in_=wv_st[:])

    # ---------------- preamble building blocks ----------------
    def tp4(bank, j0):
        # transpose chunks j0..j0+3 into xT via `bank` (bf16 view; 2 copies)
        bb = bank.bitcast(BF16)
        for p in range(2):
            j = j0 + 2 * p
            off = 256 * p
            for t in range(2):
                nc.tensor.transpose(
                    bb[0:C, off + 128 * t : off + 128 * (t + 1)],
                    xb[:, C * (j + t) : C * (j + t + 1)],
                    ident[:],
                )
            nc.vector.tensor_copy(
                out=xT[0:C, 128 * j : 128 * (j + 2)],
                in_=bb[0:C, off : off + 256],
            )

    def proj512(bank, w, dest, c0):
        # one [128,512] q/k projection through a whole scratch bank
        nc.tensor.matmul(
            bank[:, 0:512], w[:], xT[:, c0 : c0 + 512], start=True, stop=True
        )
        nc.vector.tensor_copy(out=dest[:, c0 : c0 + 512], in_=bank[:, 0:512])

    def vp8(bank, j0):
        # v projections for chunks j0..j0+7 through a whole bank
        for t in range(8):
            j = j0 + t
            nc.tensor.matmul(
                bank[:, 64 * t : 64 * (t + 1)],
                xT[:, 128 * j : 128 * (j + 1)],
                wvp[:],
                start=True, stop=True,
            )
        for p in range(4):
            j = j0 + 2 * p
            nc.vector.tensor_copy(
                out=v_all[:].rearrange("p (c f) -> p c f", f=C + 1)[
                    :, j : j + 2, 0:C
                ],
                in_=bank[:, 128 * p : 128 * (p + 1)].rearrange(
                    "p (c f) -> p c f", f=C
                ),
            )

    # ---------------- quarter 0 minimal chain (pre-stream) ----------------
    # Only what exp#0 needs: cast+transpose chunks 0-3, q/k projections for
    # cols 0-511.  Weight staging is interleaved between the two casts so it
    # is not blocked behind the second x-DMA on the DVE FIFO.
    scr_a = st_pool.tile([128, 1536], F32, tag="st", name="scr_a")
    scr_b = st_pool.tile([128, 1536], F32, tag="st", name="scr_b")
    xb_cast(0, 256)
    stage_weights()
    xb_cast(256, 256)
    tp4(scr_a[:, 0:512], 0)
    nc.tensor.matmul(
        scr_a[:, 512:1024], wqp[:], xT[:, 0:512], start=True, stop=True
    )
    nc.vector.tensor_copy(out=qT_rep[:, 0:512], in_=scr_a[:, 512:1024])
    nc.tensor.matmul(
        scr_a[:, 1024:1536], wkp[:], xT[:, 0:512], start=True, stop=True
    )
    # k copy on ScalarE: it gates exp#0 anyway, and this halves the DVE chain
    nc.scalar.copy(out=kT_rep[:, 0:512], in_=scr_a[:, 1024:1536])

    # Remaining preamble as ordered phases, each using one scratch bank;
    # drained a few per slot into the interleaved nb0/nb1 window.
    # scr_b is only safe until slot 1 claims buffer B of the st ring, so its
    # phases come first.
    phases = []

    def add(bank, fn, *args):
        phases.append((bank, fn, args))

    # slot 0 (scr_b is only safe until slot 1 claims ring buffer B):
    add(scr_b[:, 0:512], tp4, 4)
    add(scr_b[:, 512:1024], proj512, wqp, qT_rep, 512)
    add(scr_b[:, 1024:1536], proj512, wkp, kT_rep, 512)
    add(None, xb_cast, 512, 256)   # chunks 8-11; waits x q1a - keep at slot tail
    # slot 1:
    add(aux, vp8, 0)
    add(None, xb_cast, 768, 256)   # chunks 12-15
    # slot 2:
    add(aux2, tp4, 8)
    add(aux, tp4, 12)
    add(aux2, proj512, wqp, qT_rep, 1024)
    # slot 3:
    add(aux, proj512, wkp, kT_rep, 1024)
    add(aux2, proj512, wqp, qT_rep, 1536)
    add(aux, proj512, wkp, kT_rep, 1536)
    # slot 4:
    add(aux2, vp8, 8)
    add(None, xb_cast, 1024, 512)
    # slots 5-8:
    add(aux, tp4, 16)
    add(aux2, tp4, 20)
    add(aux, proj512, wqp, qT_rep, 2048)
    add(aux2, proj512, wkp, kT_rep, 2048)
    add(aux, proj512, wqp, qT_rep, 2560)
    add(aux2, proj512, wkp, kT_rep, 2560)
    add(aux, vp8, 16)
    add(None, xb_cast, 1536, 512)
    # slots 9-12:
    add(aux2, tp4, 24)
    add(aux, tp4, 28)
    add(aux2, proj512, wqp, qT_rep, 3072)
    add(aux, proj512, wkp, kT_rep, 3072)
    add(aux2, proj512, wqp, qT_rep, 3584)
    add(aux, proj512, wkp, kT_rep, 3584)
    add(aux2, vp8, 24)

    def run_phase(ph):
        bank, fn, args = ph
        if fn is xb_cast:
            fn(*args)
        else:
            fn(bank, *args)

    # phases per slot: front-load (deadlines are in the first half window)
    PHASES_PER_SLOT = [4, 2, 3, 3, 2, 2, 2, 2, 2, 2, 2, 2, 2, 2, 2]
    phase_cursor = [0]

    def emit_phases(idx):
        nph = PHASES_PER_SLOT[idx] if idx < len(PHASES_PER_SLOT) else 0
        for _ in range(nph):
            if phase_cursor[0] < len(phases):
                run_phase(phases[phase_cursor[0]])
                phase_cursor[0] += 1

    # ---------------- main loop ----------------
    # first window interleaves nb 0 and nb 1 (halves chunk deadlines and
    # needs only quarter-0 q^T); then nb 2..7 sequentially.
    slots = []
    for gi, gs in enumerate(GROUP_SIZES):
        j0 = sum(GROUP_SIZES[:gi])
        slots.append((0, gi, list(range(j0, j0 + gs))))
        slots.append((1, gi, list(range(j0, j0 + gs))))
    for nb in range(2, NB):
        for gi, gs in enumerate(GROUP_SIZES):
            j0 = sum(GROUP_SIZES[:gi])
            slots.append((nb, gi, list(range(j0, j0 + gs))))

    oaccs = {}
    pending_finales = []  # (due_idx, nb, ob_tile, k4)
    # PV work queues, chunk-granular.  Drained strictly nb-by-nb (one oacc
    # bank), in chunk order within an nb.
    pv_queues = {nb: [] for nb in range(NB)}
    pv_nb = [0]

    def emit_s_exp(nb, chunks):
        gw = NBLK * len(chunks)
        st = st_pool.tile([128, gw], F32, tag="st")
        nsl = slice(nb * NBLK, (nb + 1) * NBLK)
        for i, j in enumerate(chunks):
            nc.tensor.matmul(
                st[:, i * NBLK : (i + 1) * NBLK],
                kT_rep[32 * i : 32 * i + C8, 128 * j : 128 * (j + 1)],
                qT_rep[32 * i : 32 * i + C8, nsl],
                start=True, stop=True,
            )
        et = et_pool.tile([128, gw], BF16, tag="et")
        nc.scalar.activation(out=et[:], in_=st[:], func=AF.Exp)
        return et

    def emit_pv_budget(budget, cur_idx, min_trail=2):
        n = 0
        while n < budget and pv_nb[0] < NB:
            q = pv_queues[pv_nb[0]]
            if not q:
                break  # current nb's next chunk not produced yet
            if q[0][4] > cur_idx - min_trail:
                break  # keep PV trailing the exp stream
            nb, j, et, i, _ = q.pop(0)
            if j == 0:
                oaccs[nb] = out_pool.tile(
                    [128, NBLK], F32, tag="oacc", name=f"oacc{nb}"
                )
            nc.tensor.matmul(
                oaccs[nb][0 : C + 1, :],
                v_all[:, (C + 1) * j : (C + 1) * (j + 1)],
                et[:, i * NBLK : (i + 1) * NBLK],
                start=(j == 0), stop=(j == MCH - 1),
                skip_group_check=True,
            )
            if j == MCH - 1:
                # out^T -> bf16, then xbar DMA transpose back to [n, ch]
                # (rows 65-127 of ob are stale; the matching yt cols are
                # never read)
                ob = ob_pool.tile([128, NBLK], BF16, tag="ob")
                nc.vector.tensor_copy(
                    out=ob[0 : C + 1, :], in_=oaccs[nb][0 : C + 1, :]
                )
                yt = yt_pool.tile([128, NBLK], BF16, tag="yt")
                for t in range(4):
                    nc.sync.dma_start_transpose(
                        out=yt[:, 128 * t : 128 * (t + 1)],
                        in_=ob[:, 128 * t : 128 * (t + 1)],
                    )
                    pending_finales.append((cur_idx + 2 + t, nb, yt, t))
                pv_nb[0] += 1
            n += 1

    def emit_finale(nb, yt, k4):
        # yt[p, 128*k4 + ch]: ch 0-63 = out^T values, 64 = denominator
        rc = fin_pool.tile([128, 1], F32, tag="rc")
        nc.vector.reciprocal(rc[:], yt[:, 128 * k4 + C : 128 * k4 + C + 1])
        yo = fin_pool.tile([128, C], F32, tag="yo")
        nc.vector.tensor_scalar(
            out=yo[:], in0=yt[:, 128 * k4 : 128 * k4 + C],
            scalar1=rc[:], scalar2=gamma_sb[:],
            op0=ALU.mult, op1=ALU.mult,
        )
        ck = nb * 4 + k4
        nc.vector.tensor_add(yo[:], yo[:], x_sb[:, C * ck : C * (ck + 1)])
        nc.sync.dma_start(out=y_d[128 * ck : 128 * (ck + 1), :], in_=yo[:])

    for idx, (nb, gi, chunks) in enumerate(slots):
        while pending_finales and pending_finales[0][0] <= idx:
            _, fnb, fob, fk4 = pending_finales.pop(0)
            emit_finale(fnb, fob, fk4)
        et = emit_s_exp(nb, chunks)
        for i, j in enumerate(chunks):
            pv_queues[nb].append((nb, j, et, i, idx))
        budget = 0 if idx < 12 else 4
        emit_pv_budget(budget, idx)
        emit_phases(idx)

    # drain remaining PV work and finales
    last_idx = len(slots)
    emit_pv_budget(10**9, last_idx, min_trail=0)
    for _, fnb, fob, fk4 in pending_finales:
        emit_finale(fnb, fob, fk4)

    ctx.close()


def build_program():
    nc = bacc.Bacc("TRN2", target_bir_lowering=False, debug=False, num_devices=8)
    x_d = nc.dram_tensor("x", [N, C], F32, kind="ExternalInput").ap()
    wq_d = nc.dram_tensor("wq", [C, C8], F32, kind="ExternalInput").ap()
    bq_d = nc.dram_tensor("bq", [1, C8], F32, kind="ExternalInput").ap()
    wk_d = nc.dram_tensor("wk", [C, C8], F32, kind="ExternalInput").ap()
    bk_d = nc.dram_tensor("bk", [1, C8], F32, kind="ExternalInput").ap()
    wv_d = nc.dram_tensor("wv", [C, C], F32, kind="ExternalInput").ap()
    bv_d = nc.dram_tensor("bv", [1, C], F32, kind="ExternalInput").ap()
    gamma_d = nc.dram_tensor("gamma", [1, 1], F32, kind="ExternalInput").ap()
    y_d = nc.dram_tensor("y", [N, C], F32, kind="ExternalOutput").ap()

    io = (x_d, wq_d, bq_d, wk_d, bk_d, wv_d, bv_d, gamma_d, y_d)
    with tile.TileContext(nc) as tc:
        _body(nc, tc, io)
    nc.compile()
    return nc


_CACHE = {}


def _get_program():
    if "nc" not in _CACHE:
        _CACHE["nc"] = build_program()
    return _CACHE["nc"]


def make_in_maps(inputs):
    x = np.ascontiguousarray(np.asarray(inputs["x"], dtype=np.float32))
    wq = np.ascontiguousarray(np.asarray(inputs["wq"], dtype=np.float32))
    bq = np.asarray(inputs["bq"], dtype=np.float32).reshape(1, C8)
    wk = np.ascontiguousarray(np.asarray(inputs["wk"], dtype=np.float32))
    bk = np.asarray(inputs["bk"], dtype=np.float32).reshape(1, C8)
    wv = np.ascontiguousarray(np.asarray(inputs["wv"], dtype=np.float32))
    bv = np.asarray(inputs["bv"], dtype=np.float32).reshape(1, C)
    gamma = np.asarray(inputs["gamma"], dtype=np.float32).reshape(1, 1)
    in_maps = []
    for b in range(B):
        in_maps.append(
            {
                "x": np.ascontiguousarray(x[b].reshape(N, C)),
                "wq": wq, "bq": bq, "wk": wk, "bk": bk,
                "wv": wv, "bv": bv, "gamma": gamma,
            }
        )
    return in_maps


def run(inputs, **kwargs):
    nc = _get_program()
    res = run_bass_kernel_spmd(
        nc, make_in_maps(inputs), core_ids=list(range(B)), **kwargs
    )
    y = np.stack([res.results[b]["y"] for b in range(B)], axis=0)
    return y.reshape(B, H, W, C).astype(np.float32), res


def kernel(**inputs) -> np.ndarray:
    y, _ = run(inputs)
    return y
